# revision 11
# baseline (speedup 1.0000x reference)
"""Trainium2 Bass kernel for nn_PnP_contour_feature.

Computation (per batch image, data-parallel over 8 NeuronCores):
  mask_batch[j] = even-odd point-in-polygon rasterization of polygon idx[j]   [C,256,256]
  cnn_out      = relu((max_j mask_batch[j]) * cnn + cnn) = (max+1)*relu(cnn)  [64,256,256]

Device algorithm for the rasterization (per polygon):
  Scanline formulation. For edge e and row y the reference tests px < xint(e,y),
  i.e. each crossing edge covers a prefix of k(e,y) = clip(ceil(xint),0,256) pixels,
  and the mask row is the XOR of those prefixes. XOR-of-prefixes == suffix-XOR-scan
  of a "toggle" image T[y,b] that gets one bit flipped at bin b = k-1 per crossing
  edge. We build T as 8 packed int32 words per row via per-edge bit values
  (1 << (b&31)) masked per word (b>>5 == w) and a bitwise-XOR reduction over the
  edge axis, then unpack bits and run tensor_tensor_scan(logical_xor) along x.
  Bins are x-mirrored (bm = 256-k) so a forward scan yields the mask; the scan
  writes through a reversed access pattern to un-mirror.

All arithmetic replicates the fp32 op order of the reference exactly (verified
bit-exact in simulation): dy = y - y1; n = dy*dx; xint = n/denom + x1 (single
tensor_scalar with op0=divide, op1=add); ceil via int-cast + compare fixup.
"""

import numpy as np

H = 256
W = 256
NE = 128          # edges per polygon
CIN = 64          # cnn channels
NCORES = 8
EDCOLS = 8        # per-edge scalar columns: x1, y1, y2, dx, denom, recip, pad, pad

_PROGRAM_CACHE = {}


def _rev_last(ap_obj):
    """Return a copy of the AP with the innermost free dim reversed."""
    import concourse.bass as bass
    pairs = [list(p) for p in ap_obj.ap]
    step, n = pairs[-1]
    pairs[-1] = [-step, n]
    return bass.AP(ap_obj.tensor, ap_obj.offset + step * (n - 1), pairs)


def build_program(C, use_divide=False):
    """Build the per-core Bass/Tile program. Inputs: edata [C,128,8] f32,
    cnn [64,256,256] f32. Outputs: mask_out [C,256,256] f32, cnn_out [64,256,256] f32."""
    import concourse.bass as bass
    import concourse.bacc as bacc
    import concourse.mybir as mybir
    from concourse.tile import TileContext
    from concourse.masks import make_identity

    Alu = mybir.AluOpType
    Act = mybir.ActivationFunctionType
    f32 = mybir.dt.float32
    i32 = mybir.dt.int32

    nc = bacc.Bacc("TRN2", target_bir_lowering=False, debug=False)

    edata = nc.dram_tensor("edata", [C, NE, EDCOLS], f32, kind="ExternalInput")
    cnn = nc.dram_tensor("cnn", [CIN, H, W], f32, kind="ExternalInput")
    mask_out = nc.dram_tensor("mask_out", [C, H, W], f32, kind="ExternalOutput")
    cnn_out = nc.dram_tensor("cnn_out", [CIN, H, W], f32, kind="ExternalOutput")

    with TileContext(nc) as tc:
        from contextlib import ExitStack
        with ExitStack() as ctx:
            const = ctx.enter_context(tc.tile_pool(name="const", bufs=1))
            pnp = ctx.enter_context(tc.tile_pool(name="pnp", bufs=2))
            stepc = ctx.enter_context(tc.tile_pool(name="stepc", bufs=2))
            maskp = ctx.enter_context(tc.tile_pool(name="maskp", bufs=1))
            tp_psum = ctx.enter_context(tc.tile_pool(name="tp_psum", bufs=2, space="PSUM"))
            octp = ctx.enter_context(tc.tile_pool(name="octp", bufs=3))

            # ---------- constants ----------
            iota_i = const.tile([128, W], i32)
            nc.gpsimd.iota(iota_i[:], [[1, W]], channel_multiplier=0)
            iota_f = const.tile([128, W], f32)
            nc.vector.tensor_copy(out=iota_f[:], in_=iota_i[:])

            ones_i = const.tile([128, W], i32)
            nc.vector.memset(ones_i[:], 1)
            zeros_f = const.tile([128, W], f32)
            nc.vector.memset(zeros_f[:], 0.0)

            bitpos = const.tile([128, W], i32)
            nc.gpsimd.iota(bitpos[:], [[0, 8], [1, 32]], channel_multiplier=0)
            bitmask = const.tile([128, W], i32)
            nc.vector.tensor_tensor(out=bitmask[:], in0=ones_i[:], in1=bitpos[:],
                                    op=Alu.logical_shift_left)

            ident = const.tile([128, 128], f32)
            make_identity(nc, ident[:])

            ed = const.tile([128, C, EDCOLS], f32)
            nc.sync.dma_start(out=ed[:], in_=edata.ap().transpose([1, 0, 2]))

            # masks: one tile per y-chunk, [128, C, 256]
            mask_all = [maskp.tile([128, C, W], f32, tag=f"mask_all{chv}",
                                   name=f"mask_all{chv}") for chv in range(2)]
            # stepA per-poly outputs (mirrored toggle bins as f32), all polys
            bf_all = maskp.tile([128, C, W], f32, name="bf_all")

            cnn_ap = cnn.ap()
            cnn_out_ap = cnn_out.ap()
            GRP = 4          # channels per output tile
            NGRP = CIN // GRP

            # prefetch chunk-0 cnn tiles early (16 tiles x [128, 4, 256])
            cnn_tiles = {}
            def load_cnn(ch):
                for g in range(NGRP):
                    t = octp.tile([128, GRP, W], f32, tag="cnn_t", bufs=NGRP + 1,
                                  name=f"cnn_t_{ch}_{g}")
                    src = bass.AP(cnn_ap.tensor,
                                  cnn_ap.offset + ch * 128 * W + g * GRP * H * W,
                                  [[W, 128], [H * W, GRP], [1, W]])
                    nc.sync.dma_start(out=t[:], in_=src)
                    cnn_tiles[(ch, g)] = t

            load_cnn(0)

            # ---------- stage A: per-(edge,row) toggle bins for all polygons ----------
            for c in range(C):
                x1 = ed[:, c, 0:1]
                y1 = ed[:, c, 1:2]
                y2 = ed[:, c, 2:3]
                dx = ed[:, c, 3:4]
                dn = ed[:, c, 4:5]
                rcp = ed[:, c, 5:6]

                dy = pnp.tile([128, H], f32, tag="dy")
                nc.vector.tensor_scalar(out=dy[:], in0=iota_f[:], scalar1=y1,
                                        scalar2=None, op0=Alu.subtract)
                nmul = pnp.tile([128, H], f32, tag="nmul")
                nc.vector.tensor_scalar(out=nmul[:], in0=dy[:], scalar1=dx,
                                        scalar2=None, op0=Alu.mult)
                xint = pnp.tile([128, H], f32, tag="xint")
                if use_divide:
                    nc.vector.tensor_scalar(out=xint[:], in0=nmul[:], scalar1=dn,
                                            scalar2=x1, op0=Alu.divide, op1=Alu.add)
                else:
                    qr = pnp.tile([128, H], f32, tag="qr")
                    nc.vector.tensor_scalar(out=qr[:], in0=nmul[:], scalar1=rcp,
                                            scalar2=None, op0=Alu.mult)
                    tres = pnp.tile([128, H], f32, tag="tres")
                    nc.vector.scalar_tensor_tensor(out=tres[:], in0=qr[:], scalar=dn,
                                                   in1=nmul[:], op0=Alu.mult,
                                                   op1=Alu.subtract)
                    nq1 = pnp.tile([128, H], f32, tag="nq1")
                    nc.vector.scalar_tensor_tensor(out=nq1[:], in0=tres[:], scalar=rcp,
                                                   in1=qr[:], op0=Alu.mult,
                                                   op1=Alu.subtract)
                    nc.vector.tensor_scalar(out=xint[:], in0=nq1[:], scalar1=-1.0,
                                            scalar2=x1, op0=Alu.mult, op1=Alu.add)

                cA = pnp.tile([128, H], f32, tag="cA")
                nc.vector.tensor_scalar(out=cA[:], in0=iota_f[:], scalar1=y1,
                                        scalar2=None, op0=Alu.is_lt)
                cr = pnp.tile([128, H], f32, tag="cr")
                nc.vector.scalar_tensor_tensor(out=cr[:], in0=iota_f[:], scalar=y2,
                                               in1=cA[:], op0=Alu.is_lt,
                                               op1=Alu.logical_xor)
                cm = pnp.tile([128, H], f32, tag="cm")
                nc.vector.tensor_scalar(out=cm[:], in0=cr[:], scalar1=2e9,
                                        scalar2=-1.0, op0=Alu.mult, op1=Alu.add)
                te = pnp.tile([128, H], f32, tag="te")
                nc.vector.scalar_tensor_tensor(out=te[:], in0=xint[:], scalar=-2.0,
                                               in1=cm[:], op0=Alu.max, op1=Alu.min)
                xi = pnp.tile([128, H], i32, tag="xi")
                nc.vector.tensor_copy(out=xi[:], in_=te[:])
                gf = pnp.tile([128, H], f32, tag="gf")
                nc.vector.tensor_tensor(out=gf[:], in0=xi[:], in1=te[:], op=Alu.is_lt)
                cv = pnp.tile([128, H], f32, tag="cv")
                nc.vector.tensor_tensor(out=cv[:], in0=gf[:], in1=xi[:], op=Alu.add)
                # bm = clamp(256 - ceil, 0, 256); 256 acts as no-toggle sentinel
                bm = pnp.tile([128, H], f32, tag="bm")
                nc.vector.tensor_scalar(out=bm[:], in0=cv[:], scalar1=-1.0,
                                        scalar2=256.0, op0=Alu.mult, op1=Alu.add)
                nc.vector.tensor_scalar(out=bf_all[:, c, :], in0=bm[:], scalar1=0.0,
                                        scalar2=256.0, op0=Alu.max, op1=Alu.min)

            # ---------- per y-chunk: rasterize all polys, then fused output ----------
            for ch in range(2):
                for c in range(C):
                    btp = tp_psum.tile([128, 128], f32, tag="btp")
                    nc.tensor.transpose(out=btp[:],
                                        in_=bf_all[:, c, ch * 128:(ch + 1) * 128],
                                        identity=ident[:])
                    bi = stepc.tile([128, 128], i32, tag="bi")
                    nc.vector.tensor_copy(out=bi[:], in_=btp[:])
                    wv = stepc.tile([128, 128], i32, tag="wv")
                    nc.vector.tensor_scalar(out=wv[:], in0=bi[:], scalar1=5,
                                            scalar2=None, op0=Alu.arith_shift_right)
                    amt = stepc.tile([128, 128], i32, tag="amt")
                    nc.vector.tensor_scalar(out=amt[:], in0=bi[:], scalar1=31,
                                            scalar2=None, op0=Alu.bitwise_and)
                    vv = stepc.tile([128, 128], i32, tag="vv")
                    nc.vector.tensor_tensor(out=vv[:], in0=ones_i[:, :128], in1=amt[:],
                                            op=Alu.logical_shift_left)
                    mmt = stepc.tile([128, 8, 128], i32, tag="mmt")
                    for w in range(8):
                        nc.vector.scalar_tensor_tensor(out=mmt[:, w, :], in0=wv[:],
                                                       scalar=float(w), in1=vv[:],
                                                       op0=Alu.is_equal, op1=Alu.mult)
                    # XOR-reduce over the edge axis: log2(128)-step tensor_tensor tree
                    cur = mmt
                    width = 128
                    while width > 1:
                        half = width // 2
                        nxt = stepc.tile([128, 8, half], i32, tag=f"xr{half}",
                                         name=f"xr{half}")
                        nc.vector.tensor_tensor(out=nxt[:], in0=cur[:, :, 0:half],
                                                in1=cur[:, :, half:width],
                                                op=Alu.bitwise_xor)
                        cur = nxt
                        width = half
                    tp = cur  # [128, 8, 1]
                    ta = stepc.tile([128, W], i32, tag="ta")
                    tp_ap = tp[:, :, 0]
                    tp_rep = bass.AP(tp_ap.tensor, tp_ap.offset,
                                     [list(tp_ap.ap[0]), [1, 8], [0, 32]])
                    bm_view = bitmask[:].rearrange("p (w b) -> p w b", b=32)
                    nc.vector.tensor_tensor(out=ta[:].rearrange("p (w b) -> p w b", b=32),
                                            in0=tp_rep, in1=bm_view, op=Alu.bitwise_and)
                    tog = stepc.tile([128, W], f32, tag="tog")
                    nc.vector.tensor_scalar(out=tog[:], in0=ta[:], scalar1=0,
                                            scalar2=None, op0=Alu.not_equal)
                    nc.vector.tensor_tensor_scan(out=_rev_last(mask_all[ch][:, c, :]),
                                                 data0=tog[:], data1=zeros_f[:],
                                                 initial=0.0, op0=Alu.logical_xor,
                                                 op1=Alu.bypass)
                    nc.sync.dma_start(out=mask_out.ap()[c, ch * 128:(ch + 1) * 128, :],
                                      in_=mask_all[ch][:, c, :])

                # max over channels, +1
                maxm = const.tile([128, W], f32, tag=f"maxm{ch}", name=f"maxm{ch}")
                red_in = bass.AP(mask_all[ch][:].tensor, mask_all[ch][:].offset,
                                 [list(mask_all[ch][:].ap[0]), [1, W], [W, C]])
                nc.vector.tensor_reduce(out=maxm[:], in_=red_in,
                                        axis=mybir.AxisListType.X, op=Alu.max)
                mm1 = const.tile([128, W], f32, tag=f"mm1{ch}", name=f"mm1{ch}")
                nc.vector.tensor_scalar(out=mm1[:], in0=maxm[:], scalar1=1.0,
                                        scalar2=None, op0=Alu.add)
                mm1_b = bass.AP(mm1[:].tensor, mm1[:].offset,
                                [list(mm1[:].ap[0]), [0, GRP], [1, W]])

                # fused (max+1)*relu(cnn) for this chunk's rows
                for g in range(NGRP):
                    cnn_t = cnn_tiles.pop((ch, g))
                    relu_t = octp.tile([128, GRP, W], f32, tag="relu_t")
                    nc.scalar.activation(out=relu_t[:], in_=cnn_t[:], func=Act.Relu)
                    out_t = octp.tile([128, GRP, W], f32, tag="out_t")
                    nc.vector.tensor_tensor(out=out_t[:], in0=relu_t[:], in1=mm1_b,
                                            op=Alu.mult)
                    dst = bass.AP(cnn_out_ap.tensor,
                                  cnn_out_ap.offset + ch * 128 * W + g * GRP * H * W,
                                  [[W, 128], [H * W, GRP], [1, W]])
                    nc.sync.dma_start(out=dst, in_=out_t[:])
                if ch == 0:
                    load_cnn(1)

    nc.compile()
    return nc


def _host_inputs(contour, ct_num):
    """Per-core edata arrays: [C, 128, 8] f32 per core."""
    f = np.float32
    ct = np.asarray(ct_num).astype(np.int64)
    C = int(ct.max())
    starts = np.concatenate([[0], ct[:-1]]).astype(np.int64)
    contour = np.asarray(contour, dtype=np.float32)
    P = contour.shape[0]

    x1a = contour[:, :, 0]
    y1a = contour[:, :, 1]
    x2a = np.roll(x1a, -1, axis=1)
    y2a = np.roll(y1a, -1, axis=1)
    dxa = (x2a - x1a).astype(f)
    dena = np.where(y2a == y1a, f(1.0), (y2a - y1a).astype(f)).astype(f)
    rcpa = (f(1.0) / dena).astype(f)

    # degenerate polygon -> all-zero mask (never crosses any scanline)
    deg = np.zeros((NE, EDCOLS), dtype=f)
    deg[:, 1] = deg[:, 2] = -5.0
    deg[:, 4] = 1.0
    deg[:, 5] = 1.0

    edata_cores = []
    for i in range(NCORES):
        e = np.zeros((C, NE, EDCOLS), dtype=f)
        for j in range(C):
            if j < ct[i]:
                p = int(np.clip(starts[i] + j, 0, P - 1))
                e[j, :, 0] = x1a[p]
                e[j, :, 1] = y1a[p]
                e[j, :, 2] = y2a[p]
                e[j, :, 3] = dxa[p]
                e[j, :, 4] = dena[p]
                e[j, :, 5] = rcpa[p]
            else:
                e[j] = deg
        edata_cores.append(e)
    return edata_cores, C


def kernel(contour=None, cnn_feature=None, ct_num=None, _trace=False):
    from concourse.bass_utils import run_bass_kernel_spmd

    cnn_feature = np.ascontiguousarray(np.asarray(cnn_feature, dtype=np.float32))
    edata_cores, C = _host_inputs(contour, ct_num)

    key = (C,)
    if key not in _PROGRAM_CACHE:
        _PROGRAM_CACHE[key] = build_program(C)
    nc = _PROGRAM_CACHE[key]

    in_maps = [{"edata": edata_cores[i], "cnn": cnn_feature[i]} for i in range(NCORES)]
    res = run_bass_kernel_spmd(nc, in_maps, core_ids=list(range(NCORES)), trace=_trace)
    mask_batch = np.stack([res.results[i]["mask_out"] for i in range(NCORES)])
    cnn_out = np.stack([res.results[i]["cnn_out"] for i in range(NCORES)])
    if _trace:
        kernel.last_results = res
    return mask_batch, cnn_out
